# revision 8
# baseline (speedup 1.0000x reference)
"""Trainium2 Bass kernel for nn_DualWeightAttention (B=2, S=2048, H=2048, 16 heads).

Sharding: tensor-parallel over heads — 2 heads per core on 8 cores.
Each core computes q/k/v projections for its 2 heads, attention for those
heads (both batches), and a partial output projection against its 256-row
slice of Wo.T. The 8 partial [4096, 2048] fp16 outputs are summed on the host.

On-chip layouts (per core), default dtype fp16 (same PE rate as bf16,
8x finer mantissa; every tensor here is O(1)-scaled so range is ample):
  qT, kT  [128(d), head, B*S]  fp16  (head dim on partitions)
  v       [128(s), tile, 256]  fp16  (seq on partitions)
  scoresT [128(k), 2, q]       psum f32, 2-bank pair: two QK k-tile matmuls
  attn_u  [128(k), kt, 512]    fp16  = exp(scoresT) * exp(maskT), emitted as
                                       kt-PAIRS: one ScalarE exp over 1024
                                       cols evacuates both banks, one DVE
                                       fp16 multiply applies the mask factor
  dsum    [128(k), 8, 512]     fp16  = attn_u[:,0:8] + attn_u[:,8:16] (DVE);
                                       halves the PE's denominator matmuls
  uT      [128(d), head, S]    fp16  = ((attn_u @ v)^T) * 1/denom
  out     [128(s), 512]        fp16  = uT.T @ WoT-slice (2-head accumulate)

Softmax is unnormalized: the denominator comes from a ones-vector matmul
over dsum (8 k-tile pairs) on the PE, 1/denom via a DVE approx reciprocal,
broadcast across partitions on GpSimd, and applied while evacuating the PV
accumulator.

The phase-2 emission is a software pipeline: period i interleaves QK(i+1)
kt-pair groups with PV(i) matmuls, denominator matmuls of unit i, and the
out-projection of the q-chunk finished in period i-1; out-projection PSUM
evacuations rotate over ScalarE/DVE/GpSimd so no single engine paces the PE.
"""

import numpy as np

import concourse.mybir as mybir
import concourse.tile as tile
from concourse import bacc
from concourse.bass_utils import run_bass_kernel_spmd

P = 128
B = 2
S = 2048
H = 2048
NH = 16
HD = 128
NCORES = 8
HPC = NH // NCORES  # heads per core
DC = HPC * HD       # d-columns per core
QC = 512            # q-chunk (matmul moving free dim)
HT = H // P         # contraction tiles for projections
SCALE = 1.0 / float(np.sqrt(HD))

F32 = mybir.dt.float32
F32R = mybir.dt.float32r
# fp16 over bf16: same PE/DVE rates, 8x finer mantissa; all tensors here are
# O(1)-scaled so fp16's range is ample
BF16 = mybir.dt.float16

PROJ_DT = BF16  # hsT + wq/wk/wv
QK_DT = BF16    # qT/kT operands
OUT_DT = BF16   # uT + woT
MASK_DT = BF16
EXP = mybir.ActivationFunctionType.Exp
ADD = mybir.AluOpType.add
MULT = mybir.AluOpType.mult


def build_attention_nc(s=S):
    bs = B * s
    kt_n = s // P   # k tiles per batch
    ktp_n = kt_n // 2  # k-tile pairs
    nq = s // QC    # q chunks per batch
    st_n = s // P   # s tiles per batch (out projection)
    vt_n = bs // P  # v tiles (both batches)

    nc = bacc.Bacc("TRN2", target_bir_lowering=False, debug=False, num_devices=NCORES)
    hsT = nc.dram_tensor("hsT", [H, bs], PROJ_DT, kind="ExternalInput")
    maskT = nc.dram_tensor("maskT", [B, s, s], MASK_DT, kind="ExternalInput")
    wqT = nc.dram_tensor("wqT", [H, DC], PROJ_DT, kind="ExternalInput")
    wkT = nc.dram_tensor("wkT", [H, DC], PROJ_DT, kind="ExternalInput")
    wvT = nc.dram_tensor("wvT", [H, DC], PROJ_DT, kind="ExternalInput")
    woT = nc.dram_tensor("woT", [DC, H], OUT_DT, kind="ExternalInput")
    out = nc.dram_tensor("out", [bs, H], BF16, kind="ExternalOutput")

    hsT_r = hsT.ap().rearrange("(o p) t -> p o t", p=P)
    wq_r = wqT.ap().rearrange("(o p) d -> p o d", p=P)
    wk_r = wkT.ap().rearrange("(o p) d -> p o d", p=P)
    wv_r = wvT.ap().rearrange("(o p) d -> p o d", p=P)
    wo_r = woT.ap().rearrange("(h p) j -> p h j", p=P)
    out_r = out.ap().rearrange("(t p) j -> p t j", p=P)

    with tile.TileContext(nc) as tc:
        with (
            tc.tile_pool(name="const", bufs=1) as constp,
            tc.tile_pool(name="persist", bufs=1) as persist,
            tc.tile_pool(name="mpool", bufs=6) as mpool,
        ):
            ones_bf = constp.tile([P, 1], BF16)
            nc.vector.memset(ones_bf[:], 1.0)

            qT = persist.tile([P, HPC, bs], QK_DT)
            kT = persist.tile([P, HPC, bs], QK_DT)
            vsb = persist.tile([P, vt_n, DC], BF16)
            wo_sb = persist.tile([P, HPC, H], OUT_DT)

            units = [
                (b, qq, h)
                for b in range(B)
                for qq in range(nq)
                for h in range(HPC)
            ]
            nu = len(units)
            mslabs = {}
            KH = kt_n // 2  # mask k-tiles per half-slab

            def mask_prefetch(i):
                b, qq, h = units[i]
                if h == 0 and (b, qq) not in mslabs:
                    halves = []
                    for mh in range(2):
                        ms = mpool.tile([P, KH, QC], MASK_DT, tag="mslab")
                        nc.sync.dma_start(
                            ms[:],
                            maskT.ap()[b].rearrange("(kt p) q -> p kt q", p=P)[
                                :, mh * KH : (mh + 1) * KH,
                                qq * QC : (qq + 1) * QC,
                            ],
                        )
                        halves.append(ms)
                    mslabs[(b, qq)] = halves

            # evacuation helper: alternate DVE/ACT so neither paces the PE
            def evac(idx, dst, src):
                if idx % 2 == 0:
                    nc.scalar.copy(dst, src)
                else:
                    nc.vector.tensor_copy(dst, src)

            # ---------------- Phase 1: q/k/v projections ----------------
            with (
                tc.tile_pool(name="wpool", bufs=1) as wpool,
                tc.tile_pool(name="hpool", bufs=10) as hpool,
                tc.tile_pool(name="ppsum", bufs=2, space="PSUM") as ppsum,
                tc.tile_pool(name="vpsum", bufs=4, space="PSUM") as vpsum,
            ):
                # DMA order matters at startup: the first q-projection group
                # only needs the first wq k-tile + the first hsT k-tile, so
                # issue those at single-tile granularity and defer wk/wv/wo
                # (and the mask prefetch) behind them.
                wq_sb = wpool.tile([P, HT, DC], PROJ_DT, tag="wq")
                wk_sb = wpool.tile([P, HT, DC], PROJ_DT, tag="wk")
                wv_sb = wpool.tile([P, HT, DC], PROJ_DT, tag="wv")

                NQT = 4
                KOQ = HT // NQT  # hsT streamed as 4 quarter-K tiles per s-chunk
                for sc in range(bs // QC):
                    ssl = slice(sc * QC, (sc + 1) * QC)
                    quarters = []
                    for qf in range(NQT):
                        hst = hpool.tile([P, KOQ, QC], PROJ_DT, tag="hst")
                        if sc == 0:
                            # per-k-tile DMAs so the first matmul group only
                            # waits on 1/16 of the chunk, interleaved with the
                            # matching wq k-tile
                            for j in range(KOQ):
                                nc.sync.dma_start(
                                    wq_sb[:, qf * KOQ + j],
                                    wq_r[:, qf * KOQ + j],
                                )
                                nc.sync.dma_start(
                                    hst[:, j], hsT_r[:, qf * KOQ + j, ssl]
                                )
                        else:
                            nc.sync.dma_start(
                                hst[:], hsT_r[:, qf * KOQ : (qf + 1) * KOQ, ssl]
                            )
                        quarters.append(hst)
                    if sc == 0:
                        nc.sync.dma_start(wk_sb[:], wk_r)
                        nc.sync.dma_start(wv_sb[:], wv_r)
                        nc.sync.dma_start(wo_sb[:], wo_r)
                    if sc == 2:
                        mask_prefetch(0)
                        mask_prefetch(1)
                    if sc == 4:
                        mask_prefetch(2)

                    def hq(ko):
                        return quarters[ko // KOQ][:, ko % KOQ]

                    ev = sc  # evac engine round-robin
                    for h in range(HPC):
                        for wsb, dstT in ((wq_sb, qT), (wk_sb, kT)):
                            ps = ppsum.tile([P, QC], F32, tag="psqk")
                            for ko in range(HT):
                                nc.tensor.matmul(
                                    ps[:],
                                    wsb[:, ko, h * P : (h + 1) * P],
                                    hq(ko),
                                    start=(ko == 0),
                                    stop=(ko == HT - 1),
                                )
                            evac(ev, dstT[:, h, ssl], ps[:])
                            ev += 1
                    # v: ko-outer over 4 concurrent PSUM groups so each hsT
                    # quarter is consumed once and can be recycled early
                    psvs = []
                    for st in range(QC // P):
                        psv = vpsum.tile([P, DC], F32, tag="psv")
                        psvs.append(psv)
                    for ko in range(HT):
                        for st in range(QC // P):
                            nc.tensor.matmul(
                                psvs[st][:],
                                hq(ko)[:, st * P : (st + 1) * P],
                                wv_sb[:, ko, :],
                                start=(ko == 0),
                                stop=(ko == HT - 1),
                            )
                    for st in range(QC // P):
                        evac(ev, vsb[:, sc * (QC // P) + st, :], psvs[st][:])
                        ev += 1

            # ---------------- Phase 2: attention + output projection ----------------
            with (
                tc.tile_pool(name="apool", bufs=3) as apool,
                tc.tile_pool(name="dpool", bufs=2) as dpool,
                tc.tile_pool(name="upool", bufs=2) as upool,
                tc.tile_pool(name="rpool", bufs=2) as rpool,
                tc.tile_pool(name="opool", bufs=4) as opool,
                tc.tile_pool(name="spsum", bufs=2, space="PSUM") as spsum,
                tc.tile_pool(name="upsum", bufs=2, space="PSUM") as upsum,
                tc.tile_pool(name="dpsum", bufs=1, space="PSUM") as dpsum,
                tc.tile_pool(name="opsum", bufs=1, space="PSUM") as opsum,
            ):
                aslabs = {}
                dsums = {}
                psus = {}
                psds = {}
                uTs = {}

                def qk_pair(i, p):
                    # two QK k-tile matmuls into one 2-bank PSUM pair, then a
                    # single ScalarE exp over 1024 cols and a single fp16 DVE
                    # mask multiply (exp(s+m) = exp(s)*exp(m); the mask factor
                    # exp(m) is host-precomputed)
                    b, qq, h = units[i]
                    if p == 0:
                        asl = apool.tile([P, kt_n, QC], BF16, tag="aslab")
                        aslabs[i] = asl
                    asl = aslabs[i]
                    ms = mslabs[(b, qq)][p // (ktp_n // 2)]
                    pss = spsum.tile([P, 2, QC], F32, tag="pss")
                    for j in range(2):
                        kt = 2 * p + j
                        nc.tensor.matmul(
                            pss[:, j],
                            kT[:, h, b * s + kt * P : b * s + (kt + 1) * P],
                            qT[:, h, b * s + qq * QC : b * s + (qq + 1) * QC],
                            start=True,
                            stop=True,
                        )
                    kt0 = 2 * p
                    nc.scalar.activation(asl[:, kt0 : kt0 + 2], pss[:], EXP)
                    mo = kt0 % KH
                    nc.vector.tensor_tensor(
                        asl[:, kt0 : kt0 + 2], asl[:, kt0 : kt0 + 2],
                        ms[:, mo : mo + 2], MULT,
                    )

                def pv_part(i, kt):
                    b, qq, h = units[i]
                    asl = aslabs[i]
                    if kt == 0:
                        psu = upsum.tile([P, QC], F32, tag="psu")
                        psus[i] = psu
                    nc.tensor.matmul(
                        psus[i][:],
                        vsb[:, b * kt_n + kt, h * P : (h + 1) * P],
                        asl[:, kt],
                        start=(kt == 0),
                        stop=(kt == kt_n - 1),
                    )

                def den_l1(i):
                    # fp16 adds fold k-tile pairs so the PE's denominator
                    # matmul count halves; split DVE/GpSimd so neither paces
                    asl = aslabs[i]
                    ds = dpool.tile([P, KH, QC], BF16, tag="dsum")
                    dsums[i] = ds
                    hk = KH // 2
                    nc.vector.tensor_tensor(
                        ds[:, 0:hk], asl[:, 0:hk], asl[:, KH : KH + hk], ADD
                    )
                    nc.gpsimd.tensor_tensor(
                        ds[:, hk:KH], asl[:, hk:KH], asl[:, KH + hk : kt_n], ADD
                    )

                def den_mm(i, j):
                    if j == 0:
                        psd = dpsum.tile([1, QC], F32, tag="psd")
                        psds[i] = psd
                    nc.tensor.matmul(
                        psds[i][:],
                        ones_bf[:],
                        dsums[i][:, j],
                        start=(j == 0),
                        stop=(j == KH - 1),
                    )

                def den_fin(i):
                    # 1/denom: ~51-ULP DVE approx, then replicate across
                    # partitions on the idle GpSimd
                    dsums.pop(i)
                    recip = rpool.tile([1, QC], F32, tag="recip")
                    nc.vector.reciprocal_approx_fast(out=recip[:], in_=psds.pop(i)[:])
                    rbc = rpool.tile([P, QC], F32, tag="rbc")
                    nc.gpsimd.partition_broadcast(rbc[:], recip[:])
                    return rbc

                def finish_unit(i, rbc):
                    b, qq, h = units[i]
                    aslabs.pop(i)
                    if b not in uTs:
                        uT_new = upool.tile([P, HPC, s], OUT_DT, tag="uT", name="uT")
                        uTs[b] = uT_new
                    nc.vector.tensor_tensor(
                        uTs[b][:, h, qq * QC : (qq + 1) * QC],
                        psus.pop(i)[:],
                        rbc[:],
                        MULT,
                    )

                def outproj_group(b, qq, g, ei):
                    # one PSUM group of the out-projection with 2-head
                    # accumulation, evacuation alternating ScalarE/DVE
                    uT_b = uTs[b]
                    st = qq * (QC // P) + g // (H // QC)
                    jc = g % (H // QC)
                    pso = opsum.tile([P, QC], F32, tag="pso")
                    for h in range(HPC):
                        nc.tensor.matmul(
                            pso[:],
                            uT_b[:, h, st * P : (st + 1) * P],
                            wo_sb[:, h, jc * QC : (jc + 1) * QC],
                            start=(h == 0),
                            stop=(h == HPC - 1),
                        )
                    ot = opool.tile([P, QC], BF16, tag="ot")
                    if ei % 2 == 0:
                        nc.scalar.copy(ot[:], pso[:])
                    else:
                        nc.vector.tensor_copy(ot[:], pso[:])
                    nc.sync.dma_start(
                        out_r[:, b * st_n + st, jc * QC : (jc + 1) * QC], ot[:]
                    )

                NG = (QC // P) * (H // QC)  # out-projection groups per chunk

                # software pipeline: period i interleaves QK(i+1) pair-groups
                # with PV(i), the denominator matmuls of unit i (slots 2-5,
                # behind the DVE/GpSimd pair-folds), and up to 8
                # out-projection groups from the queue of finished chunks
                # (each chunk drains over two periods)
                for p in range(ktp_n):
                    qk_pair(0, p)
                oqueue = []  # (b, qq, g) out-projection groups due
                ei = 0       # out-projection evacuation round-robin
                for i in range(nu):
                    if i + 3 < nu:
                        mask_prefetch(i + 3)
                    den_l1(i)
                    for p in range(ktp_n):
                        if i + 1 < nu:
                            qk_pair(i + 1, p)
                        pv_part(i, 2 * p)
                        pv_part(i, 2 * p + 1)
                        if 2 <= p <= 5:
                            den_mm(i, 2 * (p - 2))
                            den_mm(i, 2 * (p - 2) + 1)
                        if oqueue:
                            outproj_group(*oqueue.pop(0), ei)
                            ei += 1
                    rbc = den_fin(i)
                    finish_unit(i, rbc)
                    b, qq, h = units[i]
                    if h == HPC - 1:
                        oqueue.extend((b, qq, g) for g in range(NG))
                # epilogue: drain the remaining out-projection groups
                for args in oqueue:
                    outproj_group(*args, ei)
                    ei += 1

    nc.compile()
    return nc


def make_in_maps(hs, mask, Wq, Wk, Wv, Wo):
    """Host-side prep: transpose/shard the full inputs into per-core maps."""
    bs = hs.shape[0] * hs.shape[1]
    proj_np = np.float16
    out_np = np.float16
    hsT = np.ascontiguousarray(hs.reshape(bs, H).T).astype(proj_np)
    maskT = np.exp(
        np.ascontiguousarray(mask[:, 0].transpose(0, 2, 1))
    ).astype(np.float16)
    in_maps = []
    for c in range(NCORES):
        sl = slice(c * DC, (c + 1) * DC)
        in_maps.append(
            {
                "hsT": hsT,
                "maskT": maskT,
                "wqT": np.ascontiguousarray((Wq[sl] * SCALE).T).astype(proj_np),
                "wkT": np.ascontiguousarray(Wk[sl].T).astype(proj_np),
                "wvT": np.ascontiguousarray(Wv[sl].T).astype(proj_np),
                "woT": np.ascontiguousarray(Wo[:, sl].T).astype(out_np),
            }
        )
    return in_maps


_NC_CACHE = {}


def get_nc(s=S):
    if s not in _NC_CACHE:
        _NC_CACHE[s] = build_attention_nc(s)
    return _NC_CACHE[s]


def run(hs, mask, Wq, Wk, Wv, Wo, trace=False, trace_kwargs=None):
    s = hs.shape[1]
    nc = get_nc(s)
    in_maps = make_in_maps(hs, mask, Wq, Wk, Wv, Wo)
    res = run_bass_kernel_spmd(
        nc,
        in_maps,
        core_ids=list(range(NCORES)),
        trace=trace,
        **(trace_kwargs or {}),
    )
    parts = np.stack([r["out"] for r in res.results])
    full = parts.astype(np.float64).sum(axis=0).astype(np.float32)
    return full.reshape(hs.shape[0], s, H), res


def kernel(hidden_states, attention_mask, Wq, Wk, Wv, Wo):
    hs = np.asarray(hidden_states, dtype=np.float32)
    mask = np.asarray(attention_mask, dtype=np.float32)
    Wq = np.asarray(Wq, dtype=np.float32)
    Wk = np.asarray(Wk, dtype=np.float32)
    Wv = np.asarray(Wv, dtype=np.float32)
    Wo = np.asarray(Wo, dtype=np.float32)
    out, _ = run(hs, mask, Wq, Wk, Wv, Wo)
    return out


# revision 14
# speedup vs baseline: 1.4023x; 1.4023x over previous
"""Trainium2 Bass kernel for nn_DualWeightAttention (B=2, S=2048, H=2048, 16 heads).

Sharding: tensor-parallel over heads — 2 heads per core on 8 cores.
Each core computes q/k/v projections for its 2 heads, attention for those
heads (both batches), and a partial output projection against its 256-row
slice of Wo.T. The 8 partial [4096, 2048] fp16 outputs are summed on the host.

On-chip layouts (per core), default dtype fp16 (same PE rate as bf16,
8x finer mantissa; every tensor here is O(1)-scaled so range is ample):
  qT, kT  [128(d), head, B*S]  fp16  (head dim on partitions)
  v       [128(s), tile, 256]  fp16  (seq on partitions)
  scoresT [128(k), 2, q]       psum f32, 2-bank pair: two QK k-tile matmuls
  attn_u  [128(k), kt, 512]    fp16  = exp(scoresT) * exp(maskT), emitted as
                                       kt-PAIRS: one ScalarE exp over 1024
                                       cols evacuates both banks, one DVE
                                       fp16 multiply applies the mask factor
  dsum    [128(k), 8, 512]     fp16  = attn_u[:,0:8] + attn_u[:,8:16] (DVE);
                                       halves the PE's denominator matmuls
  uT      [128(d), head, S]    fp16  = ((attn_u @ v)^T) * 1/denom
  out     [128(s), 512]        fp16  = uT.T @ WoT-slice (2-head accumulate)

Softmax is unnormalized: the denominator comes from a ones-vector matmul
over dsum (8 k-tile pairs) on the PE, 1/denom via a DVE approx reciprocal,
broadcast across partitions on GpSimd, and applied while evacuating the PV
accumulator.

The phase-2 emission is a software pipeline: period i interleaves QK(i+1)
kt-pair groups with PV(i) matmuls, denominator matmuls of unit i, and the
out-projection of the q-chunk finished in period i-1; out-projection PSUM
evacuations rotate over ScalarE/DVE/GpSimd so no single engine paces the PE.
"""

import numpy as np

import concourse.mybir as mybir
import concourse.tile as tile
from concourse import bacc
from concourse.bass_utils import run_bass_kernel_spmd

P = 128
B = 2
S = 2048
H = 2048
NH = 16
HD = 128
NCORES = 8
HPC = NH // NCORES  # heads per core
DC = HPC * HD       # d-columns per core
QC = 512            # q-chunk (matmul moving free dim)
HT = H // P         # contraction tiles for projections
SCALE = 1.0 / float(np.sqrt(HD))

F32 = mybir.dt.float32
F32R = mybir.dt.float32r
# fp16 over bf16: same PE/DVE rates, 8x finer mantissa; all tensors here are
# O(1)-scaled so fp16's range is ample
BF16 = mybir.dt.float16

PROJ_DT = BF16  # hsT + wq/wk/wv
QK_DT = BF16    # qT/kT operands
OUT_DT = BF16   # uT + woT
MASK_DT = BF16
EXP = mybir.ActivationFunctionType.Exp
ADD = mybir.AluOpType.add
MULT = mybir.AluOpType.mult


def build_attention_nc(s=S):
    bs = B * s
    kt_n = s // P   # k tiles per batch
    ktp_n = kt_n // 2  # k-tile pairs
    nq = s // QC    # q chunks per batch
    st_n = s // P   # s tiles per batch (out projection)
    vt_n = bs // P  # v tiles (both batches)

    nc = bacc.Bacc("TRN2", target_bir_lowering=False, debug=False, num_devices=NCORES)
    hsT = nc.dram_tensor("hsT", [H, bs], PROJ_DT, kind="ExternalInput")
    maskT = nc.dram_tensor("maskT", [B, s, s], MASK_DT, kind="ExternalInput")
    wqT = nc.dram_tensor("wqT", [H, DC], PROJ_DT, kind="ExternalInput")
    wkT = nc.dram_tensor("wkT", [H, DC], PROJ_DT, kind="ExternalInput")
    wvT = nc.dram_tensor("wvT", [H, DC], PROJ_DT, kind="ExternalInput")
    woT = nc.dram_tensor("woT", [DC, H], OUT_DT, kind="ExternalInput")
    out = nc.dram_tensor("out", [bs, H], BF16, kind="ExternalOutput")

    hsT_r = hsT.ap().rearrange("(o p) t -> p o t", p=P)
    wq_r = wqT.ap().rearrange("(o p) d -> p o d", p=P)
    wk_r = wkT.ap().rearrange("(o p) d -> p o d", p=P)
    wv_r = wvT.ap().rearrange("(o p) d -> p o d", p=P)
    wo_r = woT.ap().rearrange("(h p) j -> p h j", p=P)
    out_r = out.ap().rearrange("(t p) j -> p t j", p=P)

    with tile.TileContext(nc) as tc:
        with (
            tc.tile_pool(name="const", bufs=1) as constp,
            tc.tile_pool(name="persist", bufs=1) as persist,
            tc.tile_pool(name="mpool", bufs=6) as mpool,
        ):
            ones_bf = constp.tile([P, 1], BF16)
            nc.vector.memset(ones_bf[:], 1.0)

            qT = persist.tile([P, HPC, bs], QK_DT)
            kT = persist.tile([P, HPC, bs], QK_DT)
            vsb = persist.tile([P, vt_n, DC], BF16)
            wo_sb = persist.tile([P, HPC, H], OUT_DT)

            units = [
                (b, qq, h)
                for b in range(B)
                for qq in range(nq)
                for h in range(HPC)
            ]
            nu = len(units)
            mslabs = {}
            KH = kt_n // 2  # mask k-tiles per half-slab

            def mask_prefetch(i):
                b, qq, h = units[i]
                if h == 0 and (b, qq) not in mslabs:
                    halves = []
                    for mh in range(2):
                        ms = mpool.tile([P, KH, QC], MASK_DT, tag="mslab")
                        nc.sync.dma_start(
                            ms[:],
                            maskT.ap()[b].rearrange("(kt p) q -> p kt q", p=P)[
                                :, mh * KH : (mh + 1) * KH,
                                qq * QC : (qq + 1) * QC,
                            ],
                        )
                        halves.append(ms)
                    mslabs[(b, qq)] = halves

            # evacuation helper: alternate DVE/ACT so neither paces the PE
            def evac(idx, dst, src):
                if idx % 2 == 0:
                    nc.scalar.copy(dst, src)
                else:
                    nc.vector.tensor_copy(dst, src)

            # ---------------- Phase 1: q/k/v projections ----------------
            with (
                tc.tile_pool(name="wpool", bufs=1) as wpool,
                tc.tile_pool(name="hpool", bufs=10) as hpool,
                tc.tile_pool(name="ppsum", bufs=2, space="PSUM") as ppsum,
                tc.tile_pool(name="vpsum", bufs=4, space="PSUM") as vpsum,
            ):
                # DMA order matters at startup: the first q-projection group
                # only needs the first wq k-tile + the first hsT k-tile, so
                # issue those at single-tile granularity and defer wk/wv/wo
                # (and the mask prefetch) behind them.
                wq_sb = wpool.tile([P, HT, DC], PROJ_DT, tag="wq")
                wk_sb = wpool.tile([P, HT, DC], PROJ_DT, tag="wk")
                wv_sb = wpool.tile([P, HT, DC], PROJ_DT, tag="wv")
                # split the wq load so the very first matmul group only
                # waits on a quarter of the weights, not the whole tile
                for _wf in range(4):
                    _wsl = slice(_wf * (HT // 4), (_wf + 1) * (HT // 4))
                    nc.sync.dma_start(wq_sb[:, _wsl], wq_r[:, _wsl])

                NQT = 4
                KOQ = HT // NQT  # hsT streamed as 4 quarter-K tiles per s-chunk
                for sc in range(bs // QC):
                    ssl = slice(sc * QC, (sc + 1) * QC)
                    quarters = []
                    for qf in range(NQT):
                        hst = hpool.tile([P, KOQ, QC], PROJ_DT, tag="hst")
                        nc.sync.dma_start(
                            hst[:], hsT_r[:, qf * KOQ : (qf + 1) * KOQ, ssl]
                        )
                        quarters.append(hst)
                    if sc == 0:
                        nc.sync.dma_start(wk_sb[:], wk_r)
                        nc.sync.dma_start(wv_sb[:], wv_r)
                        nc.sync.dma_start(wo_sb[:], wo_r)
                    if sc == 2:
                        mask_prefetch(0)
                        mask_prefetch(1)
                    if sc == 4:
                        mask_prefetch(2)

                    def hq(ko):
                        return quarters[ko // KOQ][:, ko % KOQ]

                    ev = sc  # evac engine round-robin
                    for h in range(HPC):
                        for wsb, dstT in ((wq_sb, qT), (wk_sb, kT)):
                            ps = ppsum.tile([P, QC], F32, tag="psqk")
                            for ko in range(HT):
                                nc.tensor.matmul(
                                    ps[:],
                                    wsb[:, ko, h * P : (h + 1) * P],
                                    hq(ko),
                                    start=(ko == 0),
                                    stop=(ko == HT - 1),
                                )
                            evac(ev, dstT[:, h, ssl], ps[:])
                            ev += 1
                    # v: ko-outer over 4 concurrent PSUM groups so each hsT
                    # quarter is consumed once and can be recycled early
                    psvs = []
                    for st in range(QC // P):
                        psv = vpsum.tile([P, DC], F32, tag="psv")
                        psvs.append(psv)
                    for ko in range(HT):
                        for st in range(QC // P):
                            nc.tensor.matmul(
                                psvs[st][:],
                                hq(ko)[:, st * P : (st + 1) * P],
                                wv_sb[:, ko, :],
                                start=(ko == 0),
                                stop=(ko == HT - 1),
                            )
                    for st in range(QC // P):
                        evac(ev, vsb[:, sc * (QC // P) + st, :], psvs[st][:])
                        ev += 1

            # ---------------- Phase 2: attention + output projection ----------------
            with (
                tc.tile_pool(name="apool", bufs=3) as apool,
                tc.tile_pool(name="upool", bufs=2) as upool,
                tc.tile_pool(name="rpool", bufs=2) as rpool,
                tc.tile_pool(name="opool", bufs=4) as opool,
                tc.tile_pool(name="spsum", bufs=2, space="PSUM") as spsum,
                tc.tile_pool(name="upsum", bufs=2, space="PSUM") as upsum,
                tc.tile_pool(name="dpsum", bufs=1, space="PSUM") as dpsum,
                tc.tile_pool(name="opsum", bufs=1, space="PSUM") as opsum,
            ):
                aslabs = {}
                psus = {}
                psds = {}
                uTs = {}

                def qk_pair(i, p):
                    # two QK k-tile matmuls into one 2-bank PSUM pair, then a
                    # single ScalarE exp over 1024 cols and a single fp16 DVE
                    # mask multiply (exp(s+m) = exp(s)*exp(m); the mask factor
                    # exp(m) is host-precomputed)
                    b, qq, h = units[i]
                    if p == 0:
                        asl = apool.tile([P, kt_n, QC], BF16, tag="aslab")
                        aslabs[i] = asl
                    asl = aslabs[i]
                    ms = mslabs[(b, qq)][p // (ktp_n // 2)]
                    pss = spsum.tile([P, 2, QC], F32, tag="pss")
                    for j in range(2):
                        kt = 2 * p + j
                        nc.tensor.matmul(
                            pss[:, j],
                            kT[:, h, b * s + kt * P : b * s + (kt + 1) * P],
                            qT[:, h, b * s + qq * QC : b * s + (qq + 1) * QC],
                            start=True,
                            stop=True,
                        )
                    kt0 = 2 * p
                    nc.scalar.activation(asl[:, kt0 : kt0 + 2], pss[:], EXP)
                    mo = kt0 % KH
                    nc.vector.tensor_tensor(
                        asl[:, kt0 : kt0 + 2], asl[:, kt0 : kt0 + 2],
                        ms[:, mo : mo + 2], MULT,
                    )

                def pv_part(i, kt):
                    b, qq, h = units[i]
                    asl = aslabs[i]
                    if kt == 0:
                        psu = upsum.tile([P, QC], F32, tag="psu")
                        psus[i] = psu
                    nc.tensor.matmul(
                        psus[i][:],
                        vsb[:, b * kt_n + kt, h * P : (h + 1) * P],
                        asl[:, kt],
                        start=(kt == 0),
                        stop=(kt == kt_n - 1),
                    )

                def den_mm(i, j):
                    # denominator ones-matmul directly over the attention
                    # slab; the PE's 303G elem/s partition reduction beats
                    # DVE/GpSimd pre-folds, whose SBUF-port contention and
                    # queueing stalled the PE outright
                    if j == 0:
                        psd = dpsum.tile([1, QC], F32, tag="psd")
                        psds[i] = psd
                    nc.tensor.matmul(
                        psds[i][:],
                        ones_bf[:],
                        aslabs[i][:, j],
                        start=(j == 0),
                        stop=(j == kt_n - 1),
                    )

                def den_fin(i):
                    # 1/denom: ~51-ULP DVE approx, then replicate across
                    # partitions on the idle GpSimd
                    recip = rpool.tile([1, QC], F32, tag="recip")
                    nc.vector.reciprocal_approx_fast(out=recip[:], in_=psds.pop(i)[:])
                    rbc = rpool.tile([P, QC], F32, tag="rbc")
                    nc.gpsimd.partition_broadcast(rbc[:], recip[:])
                    return rbc

                def finish_unit(i, rbc):
                    b, qq, h = units[i]
                    aslabs.pop(i)
                    if b not in uTs:
                        uT_new = upool.tile([P, HPC, s], OUT_DT, tag="uT", name="uT")
                        uTs[b] = uT_new
                    nc.vector.tensor_tensor(
                        uTs[b][:, h, qq * QC : (qq + 1) * QC],
                        psus.pop(i)[:],
                        rbc[:],
                        MULT,
                    )

                def outproj_group(b, qq, g, ei):
                    # one PSUM group of the out-projection with 2-head
                    # accumulation, evacuation alternating ScalarE/DVE
                    uT_b = uTs[b]
                    st = qq * (QC // P) + g // (H // QC)
                    jc = g % (H // QC)
                    pso = opsum.tile([P, QC], F32, tag="pso")
                    for h in range(HPC):
                        nc.tensor.matmul(
                            pso[:],
                            uT_b[:, h, st * P : (st + 1) * P],
                            wo_sb[:, h, jc * QC : (jc + 1) * QC],
                            start=(h == 0),
                            stop=(h == HPC - 1),
                        )
                    ot = opool.tile([P, QC], BF16, tag="ot")
                    if ei % 2 == 0:
                        nc.scalar.copy(ot[:], pso[:])
                    else:
                        nc.vector.tensor_copy(ot[:], pso[:])
                    nc.sync.dma_start(
                        out_r[:, b * st_n + st, jc * QC : (jc + 1) * QC], ot[:]
                    )

                NG = (QC // P) * (H // QC)  # out-projection groups per chunk

                # software pipeline: period i interleaves QK(i+1) pair-groups
                # with PV(i), the denominator matmuls of unit i (slots 2-5,
                # behind the DVE/GpSimd pair-folds), and up to 8
                # out-projection groups from the queue of finished chunks
                # (each chunk drains over two periods)
                for p in range(ktp_n):
                    qk_pair(0, p)
                oqueue = []  # (b, qq, g) out-projection groups due
                ei = 0       # out-projection evacuation round-robin
                for i in range(nu):
                    if i + 3 < nu:
                        mask_prefetch(i + 3)
                    for p in range(ktp_n):
                        if i + 1 < nu:
                            qk_pair(i + 1, p)
                        pv_part(i, 2 * p)
                        pv_part(i, 2 * p + 1)
                        if 1 <= p <= 4:
                            for j in range(4 * (p - 1), 4 * p):
                                den_mm(i, j)
                        if p == 4:
                            rbc = den_fin(i)
                        if oqueue:
                            outproj_group(*oqueue.pop(0), ei)
                            ei += 1
                    finish_unit(i, rbc)
                    b, qq, h = units[i]
                    if h == HPC - 1:
                        oqueue.extend((b, qq, g) for g in range(NG))
                # epilogue: drain the remaining out-projection groups
                for args in oqueue:
                    outproj_group(*args, ei)
                    ei += 1

    nc.compile()
    return nc


def make_in_maps(hs, mask, Wq, Wk, Wv, Wo):
    """Host-side prep: transpose/shard the full inputs into per-core maps."""
    bs = hs.shape[0] * hs.shape[1]
    proj_np = np.float16
    out_np = np.float16
    hsT = np.ascontiguousarray(hs.reshape(bs, H).T).astype(proj_np)
    maskT = np.exp(
        np.ascontiguousarray(mask[:, 0].transpose(0, 2, 1))
    ).astype(np.float16)
    in_maps = []
    for c in range(NCORES):
        sl = slice(c * DC, (c + 1) * DC)
        in_maps.append(
            {
                "hsT": hsT,
                "maskT": maskT,
                "wqT": np.ascontiguousarray((Wq[sl] * SCALE).T).astype(proj_np),
                "wkT": np.ascontiguousarray(Wk[sl].T).astype(proj_np),
                "wvT": np.ascontiguousarray(Wv[sl].T).astype(proj_np),
                "woT": np.ascontiguousarray(Wo[:, sl].T).astype(out_np),
            }
        )
    return in_maps


_NC_CACHE = {}


def get_nc(s=S):
    if s not in _NC_CACHE:
        _NC_CACHE[s] = build_attention_nc(s)
    return _NC_CACHE[s]


def run(hs, mask, Wq, Wk, Wv, Wo, trace=False, trace_kwargs=None):
    s = hs.shape[1]
    nc = get_nc(s)
    in_maps = make_in_maps(hs, mask, Wq, Wk, Wv, Wo)
    res = run_bass_kernel_spmd(
        nc,
        in_maps,
        core_ids=list(range(NCORES)),
        trace=trace,
        **(trace_kwargs or {}),
    )
    parts = np.stack([r["out"] for r in res.results])
    full = parts.astype(np.float64).sum(axis=0).astype(np.float32)
    return full.reshape(hs.shape[0], s, H), res


def kernel(hidden_states, attention_mask, Wq, Wk, Wv, Wo):
    hs = np.asarray(hidden_states, dtype=np.float32)
    mask = np.asarray(attention_mask, dtype=np.float32)
    Wq = np.asarray(Wq, dtype=np.float32)
    Wk = np.asarray(Wk, dtype=np.float32)
    Wv = np.asarray(Wv, dtype=np.float32)
    Wo = np.asarray(Wo, dtype=np.float32)
    out, _ = run(hs, mask, Wq, Wk, Wv, Wo)
    return out
